# revision 6
# baseline (speedup 1.0000x reference)
"""Binary tree-LSTM (BinaryTokenTreeModel) Trainium2 kernel — v5.

Problem: complete binary tree, depth 15 (N=32767 nodes), tree-LSTM with
state size 2H=512, gates 4*2H=2048, vocab 32.  Reference processes nodes
leaves-first; node i's input state is the concat of the first H=256 dims
of its two children's states.

Strategy (8 NeuronCores):
  * Data-parallel over 8 subtrees rooted at the 8 level-3 nodes (7..14).
    Each core runs a level-synchronous scan over global levels 13..7
    (2032 nodes per core).
  * VOCAB=32 => x_proj table folded into the level matmul as a one-hot
    contraction block (K = 256+256+32 = 544).
  * Leaf states take only 32 distinct values: precomputed tables (host,
    O(32) work).  Level 13's input contraction collapses to K=96
    one-hots; the 16384 leaf output rows are a host-side gather.
  * Each chunk's 2048 gate columns land in ONE 4-bank PSUM tile, so the
    whole sigmoid span is a single ACT instruction (3 ACTs per cell).
  * Tail levels 9/8 run the critical half in the serial chain with the
    deferred-half matmuls streaming right behind (keeps the PE warm);
    level 7 ships raw gates + input c to the host.
  * Global levels 7..0 (255 nodes, 0.8% of the tree) finish on the host
    with level-batched numpy GEMMs / the shipped gates.
  * Matmul operands are float16; accumulation fp32; cell intermediates
    fp16 where no DMA depends on them.

Self-contained: hardcodes all shapes; only needs numpy + the concourse
(bass) toolchain that ships with the environment.
"""

import sys

for _p in ("/opt/trn_rl_repo", "/root/.axon_site/_ro/trn_rl_repo"):
    if _p not in sys.path:
        sys.path.append(_p)

import numpy as np

import concourse.bacc as bacc
import concourse.mybir as mybir
import concourse.tile as tile
from concourse.bass_utils import run_bass_kernel_spmd

F32 = mybir.dt.float32
F16 = mybir.dt.float16
AF = mybir.ActivationFunctionType

N_CORES = 8
N = 32767
H = 256
H2 = 512
G = 2048  # 4 * H2
V = 32
LEAF0 = (1 << 14) - 1  # 16383: first leaf node id

# Gate column order: critical half (state dims 0:256) then deferred half
# (dims 256:512); within each half [i f o g] so sigmoid spans cols 0:768
# and tanh cols 768:1024 of each 1024-wide half.
GATE_PERM = np.concatenate([
    np.arange(0, 256), np.arange(512, 768),          # i_c f_c
    np.arange(1536, 1792), np.arange(1024, 1280),    # o_c g_c
    np.arange(256, 512), np.arange(768, 1024),       # i_d f_d
    np.arange(1792, 2048), np.arange(1280, 1536),    # o_d g_d
])

# (level, nodes-per-core, output row offset in the per-core out tensor)
PLAN = [
    (13, 1024, 0), (12, 512, 1024), (11, 256, 1536), (10, 128, 1792),
    (9, 64, 1920), (8, 32, 1984),
]
# out rows: 0:1984 h of levels 13..9 (tail levels: crit cols only);
# 1984:2112 L10 odd-child c; 2112:2176 L9 odd-child c; 2176:2208 L8
# even-child c; 2208:2240 L8 odd-child c.  Deferred-half gates of levels
# 10..8 and L8's crit gates ship separately in `g2` (fp16).
OUT_ROWS = 2240
OHS_OFF = {12: 0, 11: 512, 10: 768, 9: 896, 8: 960}
OHS_W = 992
CRIT_ONLY = (10, 9)        # crit chain + inline defer cell
MERGED = (9, 8)            # tail levels sharing one stationary storage
TOFF = {9: 0, 8: 64}
TAILN = 96                 # 64 + 32

_BUILT = None  # cached (nc, input_names)
LAST_RESULT = None  # BassKernelResults of the most recent run (for test.py)


def _sigmoid(x):
    return 1.0 / (1.0 + np.exp(-x))


class _Stor:
    """Per-level stationary-input storage (filled by the child level)."""

    def __init__(self, nc, L, M):
        self.M = M
        nch = max(1, (M + 127) // 128)
        mk = lambda n, sh, dt: nc.alloc_sbuf_tensor(f"{n}_{L}", sh, dt).ap()
        self.sA0 = mk("sA0", [128, M], F16)
        self.sA1 = mk("sA1", [128, M], F16)
        self.sB0 = mk("sB0", [128, M], F16)
        self.sB1 = mk("sB1", [128, M], F16)
        self.cin = mk("cin", [min(128, M), nch * 512], F32)


def _build_program(nc):
    din = {}
    for name, shape in [
        ("wk0", [128, G]), ("wk1", [128, G]), ("wk2", [128, G]), ("wk3", [128, G]),
        ("woh", [32, G]), ("w13", [96, G]),
        ("oh3", [96, 1024]), ("ohs", [32, OHS_W]),
    ]:
        din[name] = nc.dram_tensor(name, shape, F16, kind="ExternalInput").ap()
    din["eye"] = nc.dram_tensor("eye", [128, 128], F32, kind="ExternalInput").ap()
    din["cin13"] = nc.dram_tensor("cin13", [1024, 512], F16, kind="ExternalInput").ap()
    out_d = nc.dram_tensor("out", [OUT_ROWS, 512], F32, kind="ExternalOutput").ap()
    g2_d = nc.dram_tensor("g2", [256, 1024], F16, kind="ExternalOutput").ap()

    sb = lambda n, sh: nc.alloc_sbuf_tensor(n, sh, F32).ap()
    sbh = lambda n, sh: nc.alloc_sbuf_tensor(n, sh, F16).ap()
    wk = [sbh(f"wk{i}_s", [128, G]) for i in range(4)]
    woh_s = sbh("woh_s", [32, G])
    w13_s = sbh("w13_s", [96, G])
    oh3_s = sbh("oh3_s", [96, 1024])
    ohs_s = sbh("ohs_s", [32, OHS_W])
    eye_s = sb("eye_s", [128, 128])
    cin13_s = sbh("cin13_s", [128, 8 * 512])

    stor = {L: _Stor(nc, L, M) for (L, M, _) in PLAN if L in (12, 11, 10)}
    stor[9] = _Stor(nc, 9, TAILN)  # merged storage for levels 9/8
    stor[8] = stor[9]
    TOFFS = dict(TOFF)  # feed col/row base per fed level
    TOFFS[10] = 0

    with tile.TileContext(nc) as tc:
        import contextlib

        with contextlib.ExitStack() as ctx:
            gc_pool = ctx.enter_context(
                tc.tile_pool(name="gc", bufs=2, space="PSUM"))
            gd_pool = ctx.enter_context(
                tc.tile_pool(name="gd", bufs=2, space="PSUM"))
            sig_pool = ctx.enter_context(tc.tile_pool(name="sig", bufs=4))
            cell_pool = ctx.enter_context(tc.tile_pool(name="cell", bufs=3))

            # weight / one-hot loads; L13's operands first, halves split
            # across the two HWDGE queues (sync + scalar)
            nc.sync.dma_start(w13_s[0:48], din["w13"][0:48])
            nc.scalar.dma_start(w13_s[48:96], din["w13"][48:96])
            nc.sync.dma_start(oh3_s[0:48], din["oh3"][0:48])
            nc.scalar.dma_start(oh3_s[48:96], din["oh3"][48:96])
            for k in range(8):
                (nc.scalar if k % 2 else nc.sync).dma_start(
                    cin13_s[:, k * 512:(k + 1) * 512],
                    din["cin13"][k * 128:(k + 1) * 128, :])
            for d, s in [
                (din["wk0"], wk[0]), (din["wk2"], wk[2]),
                (din["woh"], woh_s), (din["eye"], eye_s),
            ]:
                nc.sync.dma_start(s, d)
            for d, s in [
                (din["wk1"], wk[1]), (din["wk3"], wk[3]),
                (din["ohs"], ohs_s),
            ]:
                nc.scalar.dma_start(s, d)

            # HAM warm-up: junk matmuls on a memset tile need no DMA, so
            # the PE is busy from the first cycle and L13 starts warm the
            # moment its operands land
            jt = nc.alloc_sbuf_tensor("jt", [128, 512], F16).ap()
            nc.vector.memset(jt, 0.0)
            wtile = gc_pool.tile([128, 1024], F32, tag="gc")
            for _ in range(16):
                nc.tensor.matmul(wtile[0:128, 0:512], jt[:, 0:128],
                                 jt[:, 0:512], start=True, stop=True,
                                 skip_group_check=True)

            def flush_fused_tail(L, item):
                (sig, gd, cnew, hnew, P, pk, row_off, c0) = item
                sig3 = sig[0:P].rearrange("p (j c) -> p j c", j=2)
                tcc = cell_pool.tile([128, 512], F16, tag="tcc")
                nc.scalar.activation(tcc[0:P], cnew[0:P], AF.Tanh)
                tcc3 = tcc[0:P].rearrange("p (j c) -> p j c", j=2)
                hnew3 = hnew[0:P].rearrange("p (j c) -> p j c", j=2)
                nc.vector.tensor_mul(hnew3, sig3[:, :, 512:768], tcc3)
                nc.sync.dma_start(
                    out_d[row_off + c0:row_off + c0 + P, :], hnew[0:P])
                feed_parent(stor[L - 1], gd, hnew[0:P, 0:256],
                            cnew[0:P], P, pk * 64, (pk // 2) * 512)

            def feed_parent(parent, gtile, hsrc, csrc, P, base, cb,
                            crit_ap=None, defer_ap=None):
                """Write child chunk crit states into parent stationary storage.

                Transposes reuse the dead crit region of the chunk's gates
                PSUM tile.  hsrc: [P, 256] h crit; csrc: [P, >=256] c crit.
                base: column (and cin-row) offset in the parent storage;
                cb: cin column-block offset; crit_ap/defer_ap: alternative
                dests for the even/odd-child c."""
                half = P // 2
                t0 = gtile[0:128, 0:P]
                nc.tensor.transpose(t0, hsrc[:, 0:128], eye_s[0:P, 0:P])
                t1 = gtile[0:128, 512:512 + P]
                nc.tensor.transpose(t1, hsrc[:, 128:256], eye_s[0:P, 0:P])
                nc.vector.tensor_copy(parent.sA0[:, base:base + half], t0[:, 0:P:2])
                nc.vector.tensor_copy(parent.sA1[:, base:base + half], t1[:, 0:P:2])
                nc.vector.tensor_copy(parent.sB0[:, base:base + half], t0[:, 1:P:2])
                nc.vector.tensor_copy(parent.sB1[:, base:base + half], t1[:, 1:P:2])
                dr = base % 128
                if crit_ap is None:
                    nc.sync.dma_start(parent.cin[dr:dr + half, cb:cb + 256],
                                      csrc[0:P:2, 0:256])
                else:
                    nc.sync.dma_start(crit_ap, csrc[0:P:2, 0:256])
                if defer_ap is None:
                    nc.sync.dma_start(parent.cin[dr:dr + half, cb + 256:cb + 512],
                                      csrc[1:P:2, 0:256])
                else:
                    nc.sync.dma_start(defer_ap, csrc[1:P:2, 0:256])

            def emit_mms(gtile, lhs_tiles, ws, col0, P, woff=0):
                """k-outer accumulation of one 1024-col gate block."""
                nk = len(lhs_tiles)
                for k in range(nk):
                    for b in range(2):
                        nc.tensor.matmul(
                            gtile[0:P, col0 + b * 512:col0 + (b + 1) * 512],
                            lhs_tiles[k],
                            ws[k][:, woff + col0 + b * 512:
                                  woff + col0 + (b + 1) * 512],
                            start=(k == 0), stop=(k == nk - 1),
                            skip_group_check=True)

            for (L, M, row_off) in PLAN:
                lvl_scope = nc.named_scope(f"L{L:02d}")
                lvl_scope.__enter__()
                nch = max(1, (M + 127) // 128)
                fused = M >= 256
                feeds = []
                for pk in range(nch):
                    P = min(128, M - pk * 128)
                    c0 = pk * 128
                    gc = gc_pool.tile([128, 1024], F32, tag="gc")
                    gd = gd_pool.tile([128, 1024], F32, tag="gd")
                    # HAM keep-warm junk: streams while the real mms wait
                    # on feeds/PSUM; start=True of the real mms clears.
                    njunk = 4 if L == 13 else (10 if L in CRIT_ONLY or L == 8
                                               else 0)
                    for _ in range(njunk):
                        nc.tensor.matmul(gc[0:128, 0:512], w13_s[:, 0:128],
                                         w13_s[:, 0:512], start=True,
                                         stop=True, skip_group_check=True)
                    if L == 13:
                        lhs_tiles = [oh3_s[:, c0:c0 + P]]
                        ws = [w13_s]
                        cin_ap = cin13_s[0:P, pk * 512:(pk + 1) * 512]
                    else:
                        st = stor[L]
                        s0 = TOFF[L] if L in MERGED else c0
                        oh_ap = ohs_s[:, OHS_OFF[L] + c0:OHS_OFF[L] + c0 + P]
                        lhs_tiles = [st.sA0[:, s0:s0 + P], st.sA1[:, s0:s0 + P],
                                     st.sB0[:, s0:s0 + P], st.sB1[:, s0:s0 + P],
                                     oh_ap]
                        ws = wk + [woh_s]
                        if L == 9:
                            cin_ap = st.cin[0:P, 0:512]
                        elif L == 8:
                            cin_ap = None
                        else:
                            cin_ap = st.cin[0:P, pk * 512:(pk + 1) * 512]
                    emit_mms(gc, lhs_tiles, ws, 0, P)
                    emit_mms(gd, lhs_tiles, ws, 0, P, woff=1024)

                    if L == 8:
                        # ship raw gates + input c; the host runs this cell
                        gt8 = cell_pool.tile([32, 2048], F16, tag="gt8")
                        nc.vector.tensor_copy(gt8[0:32, 0:1024], gc[0:32])
                        nc.vector.tensor_copy(gt8[0:32, 1024:2048], gd[0:32])
                        nc.sync.dma_start(g2_d[192:224], gt8[0:32, 0:1024])
                        nc.sync.dma_start(g2_d[224:256], gt8[0:32, 1024:2048])
                        lvl_scope.__exit__(None, None, None)
                        continue

                    cnew = cell_pool.tile([128, 512], F32)
                    hnew = cell_pool.tile([128, 512], F32)

                    if fused:
                        cin3 = cin_ap.rearrange("p (j c) -> p j c", j=2)
                        sig = sig_pool.tile([128, 1536], F16)
                        sig3 = sig[0:P].rearrange("p (j c) -> p j c", j=2)
                        tg = cell_pool.tile([128, 512], F16)
                        tg3 = tg[0:P].rearrange("p (j c) -> p j c", j=2)
                        # crit-half ACTs first: frees gc for chunk k+2's mms
                        nc.scalar.activation(sig[0:P, 0:768], gc[0:P, 0:768],
                                             AF.Sigmoid)
                        nc.scalar.activation(tg[0:P, 0:256], gc[0:P, 768:1024],
                                             AF.Tanh)
                        nc.scalar.activation(sig[0:P, 768:1536], gd[0:P, 0:768],
                                             AF.Sigmoid)
                        nc.scalar.activation(tg[0:P, 256:512], gd[0:P, 768:1024],
                                             AF.Tanh)
                        prod = cell_pool.tile([128, 512], F16)
                        prod3 = prod[0:P].rearrange("p (j c) -> p j c", j=2)
                        nc.vector.tensor_mul(prod3, sig3[:, :, 0:256], tg3)
                        fc = cell_pool.tile([128, 512], F16)
                        fc3 = fc[0:P].rearrange("p (j c) -> p j c", j=2)
                        nc.vector.tensor_mul(fc3, sig3[:, :, 256:512], cin3)
                        nc.vector.tensor_add(cnew[0:P], fc[0:P], prod[0:P])
                        # tcc/hnew/feed of the PREVIOUS chunk go here: the
                        # ACT queue then never idles waiting for this
                        # chunk's cnew (1-deep software pipeline)
                        if feeds:
                            flush_fused_tail(L, feeds.pop())
                        feeds.append((sig, gd, cnew, hnew, P, pk, row_off, c0))
                    else:  # crit-only level: minimum-latency crit half
                        tgc = cell_pool.tile([128, 256], F16, tag="tgc")
                        nc.scalar.activation(tgc[0:P], gc[0:P, 768:1024],
                                             AF.Tanh)
                        sigc = cell_pool.tile([128, 768], F16, tag="sigc")
                        nc.scalar.activation(sigc[0:P], gc[0:P, 0:768],
                                             AF.Sigmoid)
                        prodc = cell_pool.tile([128, 256], F16, tag="prodc")
                        nc.vector.tensor_mul(prodc[0:P], sigc[0:P, 0:256],
                                             tgc[0:P])
                        fcc = cell_pool.tile([128, 256], F16, tag="fcc")
                        nc.vector.tensor_mul(fcc[0:P], sigc[0:P, 256:512],
                                             cin_ap[:, 0:256])
                        nc.vector.tensor_add(cnew[0:P, 0:256], fcc[0:P],
                                             prodc[0:P])
                        tccc = cell_pool.tile([128, 256], F16, tag="tccc")
                        nc.scalar.activation(tccc[0:P], cnew[0:P, 0:256],
                                             AF.Tanh)
                        nc.vector.tensor_mul(hnew[0:P, 0:256],
                                             sigc[0:P, 512:768], tccc[0:P])
                        if L == 9:
                            feed_parent(stor[8], gc, hnew[0:P, 0:256],
                                        cnew[0:P], P, TOFFS[8], 0,
                                        crit_ap=out_d[2176:2208, 0:256],
                                        defer_ap=out_d[2208:2240, 0:256])
                        else:
                            # L10 feeds L9; its nodes' odd-child c (the
                            # host defer-cell input) ships from storage
                            nc.sync.dma_start(out_d[1984:2112, 0:256],
                                              st.cin[0:128, 256:512])
                            feed_parent(stor[L - 1], gc, hnew[0:P, 0:256],
                                        cnew[0:P], P, TOFFS[L - 1], 0,
                                        defer_ap=out_d[2112:2176, 0:256])
                        nc.sync.dma_start(
                            out_d[row_off:row_off + P, 0:256],
                            hnew[0:P, 0:256])
                        # deferred-half gates ship to the host (fp16);
                        # the mms above still keep the PE warm
                        gsh = cell_pool.tile([128, 1024], F16, tag="gsh")
                        nc.vector.tensor_copy(gsh[0:P], gd[0:P])
                        g2row = 0 if L == 10 else 128
                        nc.sync.dma_start(g2_d[g2row:g2row + P], gsh[0:P])

                while feeds:
                    flush_fused_tail(L, feeds.pop())
                if L != 8:
                    lvl_scope.__exit__(None, None, None)

    nc.compile()
    return [k for k in din]


def _get_built():
    global _BUILT
    if _BUILT is None:
        nc = bacc.Bacc("TRN2", target_bir_lowering=False, debug=False,
                       num_devices=N_CORES)
        names = _build_program(nc)
        _BUILT = (nc, names)
    return _BUILT


def kernel(types, a_idx, b_idx, emb, W_ih, W_hh, b_ih, b_hh):
    global LAST_RESULT
    types = np.asarray(types, np.int32)
    emb = np.asarray(emb, np.float32)
    W_ih = np.asarray(W_ih, np.float32)
    W_hh = np.asarray(W_hh, np.float32)
    b = np.asarray(b_ih, np.float32) + np.asarray(b_hh, np.float32)

    # ---- host weight reparameterization (O(V), no O(N) arithmetic) ----
    XT = (W_ih @ emb.T + b[:, None]).astype(np.float32)          # [2048, 32]
    c_leaf = _sigmoid(XT[0:512]) * np.tanh(XT[1024:1536])        # [512, 32]
    h_leaf = _sigmoid(XT[1536:2048]) * np.tanh(c_leaf)           # [512, 32]
    M_A = W_hh[:, 0:256] @ h_leaf[0:256]                         # [2048, 32]
    M_B = W_hh[:, 256:512] @ h_leaf[0:256]
    w13 = np.ascontiguousarray(
        np.vstack([M_A.T, M_B.T, XT.T])[:, GATE_PERM], np.float16)
    cl256 = np.ascontiguousarray(c_leaf[0:256].T)  # [32, 256]
    W_augT = np.vstack([W_hh.T, XT.T])[:, GATE_PERM]             # [544, 2048]
    wk = [np.ascontiguousarray(W_augT[i * 128:(i + 1) * 128], np.float16)
          for i in range(4)]
    woh = np.ascontiguousarray(W_augT[512:544], np.float16)
    eye = np.eye(128, dtype=np.float32)

    in_maps = []
    for j in range(N_CORES):
        # level 13: one-hots of (left-leaf, right-leaf, self) types
        base13 = (1 << 13) - 1 + j * 1024
        n = np.arange(base13, base13 + 1024)
        oh3 = np.zeros((96, 1024), np.float16)
        m = np.arange(1024)
        oh3[types[2 * n + 1], m] = 1.0
        oh3[32 + types[2 * n + 2], m] = 1.0
        oh3[64 + types[n], m] = 1.0
        cin13 = np.concatenate(
            [cl256[types[2 * n + 1]], cl256[types[2 * n + 2]]],
            axis=1).astype(np.float16)
        ohs = np.zeros((32, OHS_W), np.float16)
        for L in range(12, 7, -1):
            mm = 1 << (L - 3)
            basel = (1 << L) - 1 + j * mm
            off = OHS_OFF[L]
            ohs[types[basel:basel + mm], off + np.arange(mm)] = 1.0
        in_maps.append({
            "wk0": wk[0], "wk1": wk[1], "wk2": wk[2], "wk3": wk[3],
            "woh": woh, "w13": w13, "cin13": cin13,
            "oh3": oh3, "ohs": ohs, "eye": eye,
        })

    nc, _ = _get_built()
    res = run_bass_kernel_spmd(nc, in_maps, core_ids=list(range(N_CORES)))
    LAST_RESULT = res

    out = np.empty((N, H2), np.float32)
    for j in range(N_CORES):
        r = res.results[j]["out"]
        off = 0
        for L in range(13, 8, -1):
            mm = 1 << (L - 3)
            basel = (1 << L) - 1 + j * mm
            out[basel:basel + mm] = r[off:off + mm]
            off += mm
    out[LEAF0:] = h_leaf.T[types[LEAF0:]]

    # deferred-half cells of levels 10..9 and the full level-8 cell run
    # on the host from shipped raw gates (gate order i f o g per half)
    Hc = np.zeros((511, H), np.float32)
    Cc = np.zeros((511, H), np.float32)

    def defer_cell(gd_, cb_):
        c_d = (_sigmoid(gd_[:, 256:512]) * cb_
               + _sigmoid(gd_[:, 0:256]) * np.tanh(gd_[:, 768:1024]))
        return _sigmoid(gd_[:, 512:768]) * np.tanh(c_d)

    for j in range(N_CORES):
        r = res.results[j]["out"]
        g2 = res.results[j]["g2"].astype(np.float32)
        ids10 = 1023 + 128 * j + np.arange(128)
        out[ids10, 256:512] = defer_cell(g2[0:128], r[1984:2112, 0:256])
        ids9 = 511 + 64 * j + np.arange(64)
        out[ids9, 256:512] = defer_cell(g2[128:192], r[2112:2176, 0:256])
        ids = 255 + 32 * j + np.arange(32)
        gc8 = g2[192:224]
        gd8 = g2[224:256]
        ca = r[2176:2208, 0:256]
        cb = r[2208:2240, 0:256]
        c_c = (_sigmoid(gc8[:, 256:512]) * ca
               + _sigmoid(gc8[:, 0:256]) * np.tanh(gc8[:, 768:1024]))
        h_c = _sigmoid(gc8[:, 512:768]) * np.tanh(c_c)
        out[ids, 0:256] = h_c
        out[ids, 256:512] = defer_cell(gd8, cb)
        Hc[ids] = h_c
        Cc[ids] = c_c

    # global levels 7..0 (255 nodes) on host, level-batched numpy
    for L in range(7, -1, -1):
        ids = np.arange((1 << L) - 1, (1 << (L + 1)) - 1)
        a, bb = 2 * ids + 1, 2 * ids + 2
        hin = np.concatenate([Hc[a], Hc[bb]], axis=1)      # [M, 512]
        cin = np.concatenate([Cc[a], Cc[bb]], axis=1)
        gates = XT[:, types[ids]].T + hin @ W_hh.T          # [M, 2048]
        ig, fg, gg, og = np.split(gates, 4, axis=1)
        c_new = _sigmoid(fg) * cin + _sigmoid(ig) * np.tanh(gg)
        h_new = _sigmoid(og) * np.tanh(c_new)
        out[ids] = h_new
        Hc[ids] = h_new[:, 0:256]
        Cc[ids] = c_new[:, 0:256]
    return out
